# revision 35
# baseline (speedup 1.0000x reference)
"""MoE layer (top-2 of 8 experts, capacity 1229) on 8 Trainium2 NeuronCores.

Expert parallelism: core c owns expert c's FFN. The router (gate matmul +
top-2 + renormalized gates) is replicated on every core in fp32 (top-2
selection margins ~1.1e-5 require exact logits); it runs with Wg as the
PE-stationary operand and x^T streaming as the moving operand, producing
logits^T [8, T] which is PE-transposed back to token-major tiles.
Dispatch is fully on-chip: per-assignment slot positions via a
triangular-matmul cumsum, then a one-hot-matmul compaction builds the
per-slot (token, gate, occupancy) table — OH[p, s] = (St[p, c] == s+1)
from DVE is_equal in fp16 (St clamped to 2040 so all values stay
fp16-exact; positions > capacity match nothing, so overflow drops fall
out), then PE matmuls accumulate datc_c^T @ OH_c into a [4, slot] PSUM
table. Token ids ride as (hi, lo) base-64 digits so fp16 stays exact.
Tokens are gathered by row (indirect DMA, bf16) and transposed to
[d, slot] via the DMA XBAR transpose. The FFN runs bf16 matmuls; h stays
SBUF-resident so matmul2 accumulates over all 32 f-tiles at once, in
y[slot, d] orientation (h tiles stationary) so no output transpose is
needed; b2 is added via a K=1 ones-row matmul and the gate applied as a
per-partition ACT scale. Matmuls that reuse the previous stationary set
InstMatmult.ldweights=False to skip redundant PE weight loads. Each core
scatters its gate-scaled rows to its partial output; the host sums the 8
partials.
"""
import contextlib
import sys

sys.path.insert(0, "/opt/trn_rl_repo")

import ml_dtypes
import numpy as np

import concourse.bass as bass
import concourse.bacc as bacc
import concourse.mybir as mybir
from concourse.bass import IndirectOffsetOnAxis
from concourse.bass_utils import run_bass_kernel_spmd
from concourse.tile import TileContext

FP = mybir.dt.float32
BF = mybir.dt.bfloat16
F16 = mybir.dt.float16
I32 = mybir.dt.int32
U32 = mybir.dt.uint32
AF = mybir.ActivationFunctionType
OP = mybir.AluOpType

T, D, E, F = 4096, 1024, 8, 4096
CAP = 1229          # ceil(1.2 * T * 2 / 8)
CP = 1280           # padded slots (10 tiles)
NT = T // 128       # 32 token tiles
ND = D // 128       # 8 d tiles
NF = F // 32 // 4   # 32 f tiles
NS = CP // 128      # 10 slot tiles
KT = 64             # assignment chunks (2*T/128)
NCH = 8             # router token chunks of 512
OCH = [(0, 512), (512, 512), (1024, CAP - 1024)]   # one-hot windows (<=CAP)
CCH = [(0, 512), (512, 512), (1024, 208)]          # mm1 slot chunks (>=CAP)
DCH = [(0, 512), (512, 512)]                       # mm2 d chunks


def _no_ldw(mm):
    mm.ins.ldweights = False
    return mm


def build_module():
    nc = bacc.Bacc(None, target_bir_lowering=False, debug=False)
    xT = nc.dram_tensor("xT", [D, T], FP, kind="ExternalInput")
    xh = nc.dram_tensor("xh", [T, D], BF, kind="ExternalInput")
    wg = nc.dram_tensor("wg", [D, E], FP, kind="ExternalInput")
    bgc = nc.dram_tensor("bgc", [E, 1], FP, kind="ExternalInput")
    w1 = nc.dram_tensor("w1", [D, F], BF, kind="ExternalInput")
    b1t = nc.dram_tensor("b1t", [128, NF], FP, kind="ExternalInput")
    w2 = nc.dram_tensor("w2", [F, D], BF, kind="ExternalInput")
    b2r = nc.dram_tensor("b2r", [1, D], BF, kind="ExternalInput")
    tri = nc.dram_tensor("tri", [128, 128], FP, kind="ExternalInput")
    tx64 = nc.dram_tensor("tx64", [64, 64], FP, kind="ExternalInput")
    ident = nc.dram_tensor("ident", [128, 128], FP, kind="ExternalInput")
    onesc = nc.dram_tensor("onesc", [128, 1], FP, kind="ExternalInput")
    onesb = nc.dram_tensor("onesb", [1, 128], BF, kind="ExternalInput")
    eidr = nc.dram_tensor("eidr", [128, 1], FP, kind="ExternalInput")
    iota = nc.dram_tensor("iota", [128, CAP], FP, kind="ExternalInput")
    datc = nc.dram_tensor("datc", [128, KT * 4], BF, kind="ExternalInput")
    outd = nc.dram_tensor("out", [T, D], FP, kind="ExternalOutput")

    xc_view = xT.rearrange("(dt p) t -> p dt t", p=128)
    w1r = w1.rearrange("(dt p) (ft j) -> ft p dt j", p=128, j=128)
    w2r = w2.rearrange("(ft p) d -> ft p d", p=128)
    wgr = wg.rearrange("(dt p) e -> p dt e", p=128)

    with TileContext(nc) as tc, contextlib.ExitStack() as ctx:
        cpool = ctx.enter_context(tc.tile_pool(name="consts", bufs=1))
        rpool = ctx.enter_context(tc.tile_pool(name="router", bufs=1))

        # sync queue: router weights then xT chunks (feeds the PE first)
        wg_s = cpool.tile([128, ND, E], FP, tag="wg")
        nc.sync.dma_start(wg_s, wgr[:, :, :])
        id_s = cpool.tile([128, 128], FP, tag="ident")
        nc.sync.dma_start(id_s, ident[:, :])
        # scalar queue: dispatch consts (needed ~60us in), then resident W2
        bgc_s = cpool.tile([E, 1], FP, tag="bgc")
        nc.scalar.dma_start(bgc_s, bgc[:, :])
        tri_s = cpool.tile([128, 128], FP, tag="tri")
        nc.scalar.dma_start(tri_s, tri[:, :])
        tx64_s = cpool.tile([64, 64], FP, tag="tx64")
        nc.scalar.dma_start(tx64_s, tx64[:, :])
        ones_s = cpool.tile([128, 1], FP, tag="ones")
        nc.scalar.dma_start(ones_s, onesc[:, :])
        onesb_s = cpool.tile([1, 128], BF, tag="onesb")
        nc.scalar.dma_start(onesb_s, onesb[:, :])
        eid_s = cpool.tile([128, 1], FP, tag="eidr")
        nc.scalar.dma_start(eid_s, eidr[:, :])
        iota_s = cpool.tile([128, CAP], FP, tag="iota")
        datc_s = cpool.tile([128, KT * 4], BF, tag="datc")
        b1t_s = cpool.tile([128, NF], FP, tag="b1t")
        b2r_s = cpool.tile([1, D], BF, tag="b2r")
        w2_s = cpool.tile([128, NF, D], BF, tag="w2")

        logits = rpool.tile([128, NT * E], FP, tag="logits")
        vals = rpool.tile([128, NT * E], FP, tag="vals")
        idxu = rpool.tile([128, NT * E], U32, tag="idxu")
        idxf = rpool.tile([128, NT * E], FP, tag="idxf")

        # ---- router: logits^T[e, t] = Wg^T @ x^T in fp32, then transpose ----
        with tc.tile_pool(name="xcp", bufs=3) as xcp, tc.tile_pool(
            name="ps8", bufs=2, space="PSUM"
        ) as ps8p, tc.tile_pool(name="ls8p", bufs=3) as ls8p, tc.tile_pool(
            name="pstr", bufs=2, space="PSUM"
        ) as pstr:
            for ch in range(NCH):
                xc = xcp.tile([128, ND, 512], FP, tag="xc")
                if ch == 0:
                    for k in range(4):
                        q = nc.sync if k % 2 == 0 else nc.scalar
                        q.dma_start(
                            xc[:, 2 * k : 2 * k + 2, :],
                            xc_view[:, 2 * k : 2 * k + 2, 0:512])
                else:
                    nc.sync.dma_start(
                        xc[:, 0:4, :], xc_view[:, 0:4, ch * 512 : (ch + 1) * 512])
                    nc.scalar.dma_start(
                        xc[:, 4:8, :], xc_view[:, 4:8, ch * 512 : (ch + 1) * 512])
                ps8 = ps8p.tile([E, 512], FP, tag="l8")
                for dt in range(ND):
                    nc.tensor.matmul(
                        ps8,
                        lhsT=wg_s[:, dt : dt + 1, :],
                        rhs=xc[:, dt : dt + 1, :],
                        start=(dt == 0),
                        stop=(dt == ND - 1),
                    )
                ls8 = ls8p.tile([E, 512], FP, tag="ls8")
                nc.scalar.activation(ls8, ps8, AF.Identity, bias=bgc_s, scale=1.0)
                for j in range(4):
                    tt = ch * 4 + j
                    pt8 = pstr.tile([128, E], FP, tag="t8")
                    nc.tensor.transpose(
                        pt8, ls8[:, j * 128 : (j + 1) * 128], id_s[0:E, 0:E])
                    lsl = logits[:, tt * E : (tt + 1) * E]
                    nc.vector.tensor_copy(lsl, pt8)
                    nc.vector.max(vals[:, tt * E : (tt + 1) * E], lsl)
                    nc.vector.max_index(idxu[:, tt * E : (tt + 1) * E],
                                        vals[:, tt * E : (tt + 1) * E], lsl)
        nc.vector.tensor_copy(idxf, idxu)

        # dispatch consts + resident FFN weights/biases: issued after the
        # router so the router's xT chunk streams get the queues first
        nc.scalar.dma_start(iota_s, iota[:, :])
        nc.scalar.dma_start(datc_s, datc[:, :])
        nc.scalar.dma_start(b1t_s, b1t[:, :])
        nc.scalar.dma_start(b2r_s, b2r[:, :])
        for ft in range(NF):
            nc.scalar.dma_start(w2_s[:, ft : ft + 1, :], w2r[ft : ft + 1])

        # gates: g2 = sigmoid(l2 - l1) = 0.5 + 0.5*tanh(0.5*(l2-l1)); g1 = 1-g2
        diff = rpool.tile([128, NT], FP, tag="diff")
        nc.vector.tensor_sub(diff, vals[:, 1::E], vals[:, 0::E])
        g2t = rpool.tile([128, NT], FP, tag="g2")
        nc.scalar.activation(g2t, diff, AF.Tanh, bias=0.0, scale=0.5)
        nc.vector.tensor_scalar(g2t, g2t, 0.5, 0.5, op0=OP.mult, op1=OP.add)
        g1t = rpool.tile([128, NT], FP, tag="g1")
        nc.vector.tensor_scalar(g1t, g2t, -1.0, 1.0, op0=OP.mult, op1=OP.add)

        # runtime gate columns into the (hi, lo, gate, one) chunk table
        dat = rpool.tile([128, KT * 4], BF, tag="dat")
        nc.vector.tensor_copy(dat, datc_s)
        nc.vector.tensor_copy(dat[:, 2 : NT * 4 : 4], g1t)
        nc.vector.tensor_copy(dat[:, NT * 4 + 2 :: 4], g2t)

        # ------------- dispatch: positions via cumsum matmuls -------------
        mpool = ctx.enter_context(tc.tile_pool(name="main", bufs=1))
        buft = mpool.tile([128, ND, CP], BF, tag="buft")
        sinfoT = rpool.tile([128, NS, 4], FP, tag="sinfoT")
        with tc.tile_pool(name="psc", bufs=1, space="PSUM") as psc, tc.tile_pool(
            name="ptd", bufs=1, space="PSUM"
        ) as ptd, tc.tile_pool(name="ohp", bufs=6) as ohp, tc.tile_pool(
            name="pst", bufs=2, space="PSUM"
        ) as pst:
            me = rpool.tile([128, 64], FP, tag="me")
            nc.vector.tensor_tensor(
                out=me[:, 0:NT], in0=idxf[:, 0::E],
                in1=eid_s.to_broadcast([128, NT]), op=OP.is_equal)
            nc.vector.tensor_tensor(
                out=me[:, NT:64], in0=idxf[:, 1::E],
                in1=eid_s.to_broadcast([128, NT]), op=OP.is_equal)
            ps_cs = psc.tile([128, 64], FP, tag="cs")
            nc.tensor.matmul(ps_cs, lhsT=tri_s, rhs=me, start=True, stop=False)
            ps_col = pst.tile([64, 1], FP, tag="col", bufs=1)
            nc.tensor.matmul(ps_col, lhsT=me, rhs=ones_s, start=True, stop=True)
            colv = rpool.tile([64, 1], FP, tag="colv")
            nc.vector.tensor_copy(colv, ps_col)
            colb = rpool.tile([64, 128], FP, tag="colb")
            nc.vector.tensor_copy(colb, colv.to_broadcast([64, 128]))
            # accumulate per-chunk base offsets onto the intra-chunk cumsum
            nc.tensor.matmul(ps_cs, lhsT=colb, rhs=tx64_s, start=False, stop=True)
            St = rpool.tile([128, 64], FP, tag="St")
            # mask: only matching assignments carry a slot position
            nc.vector.tensor_mul(St, ps_cs, me)

            # one-hot compaction: psum[4, slot] += datc_c^T @ (St_c == iota)
            ptds = [ptd.tile([4, 512], FP, tag=f"td{i}", name=f"td{i}")
                    for i in range(3)]
            for c in range(KT):
                oh = ohp.tile([128, CAP], BF, tag="oh")
                nc.vector.tensor_scalar(
                    oh, iota_s, St[:, c : c + 1], None, op0=OP.is_equal)
                for i, (c0, cw) in enumerate(OCH):
                    nc.tensor.matmul(
                        ptds[i][:, 0:cw],
                        lhsT=dat[:, 4 * c : 4 * c + 4],
                        rhs=oh[:, c0 : c0 + cw],
                        start=(c == 0),
                        stop=(c == KT - 1),
                    )
            sts = rpool.tile([4, CP], FP, tag="sts")
            nc.vector.memset(sts, 0.0)
            for i, (c0, cw) in enumerate(OCH):
                nc.vector.tensor_copy(sts[:, c0 : c0 + cw], ptds[i][:, 0:cw])
            # per slot tile: transpose [4, 128] -> [128, 4], extract the token
            # id, and kick off its gather + XBAR transpose immediately
            tid_f = rpool.tile([128, NS], FP, tag="tidf")
            tid_i = rpool.tile([128, NS], I32, tag="tidi")
            with tc.tile_pool(name="xg", bufs=3) as xgp:
                for a in range(NS):
                    pt = pst.tile([128, 4], FP, tag="t")
                    nc.tensor.transpose(
                        pt, sts[:, a * 128 : (a + 1) * 128], id_s[0:4, 0:4])
                    nc.vector.tensor_copy(sinfoT[:, a : a + 1, :], pt)
                    ta = tid_f[:, a : a + 1]
                    nc.vector.tensor_scalar(
                        ta, sinfoT[:, a : a + 1, 0:1].rearrange(
                            "p a c -> p (a c)"), 64.0, None, op0=OP.mult)
                    nc.vector.tensor_add(
                        ta, ta, sinfoT[:, a : a + 1, 1:2].rearrange(
                            "p a c -> p (a c)"))
                    nc.vector.tensor_copy(tid_i[:, a : a + 1], ta)
                    xg = xgp.tile([128, D], BF, tag="xg")
                    nc.gpsimd.indirect_dma_start(
                        out=xg,
                        out_offset=None,
                        in_=xh[:, :],
                        in_offset=IndirectOffsetOnAxis(
                            ap=tid_i[:, a : a + 1], axis=0),
                    )
                    q = nc.sync if a % 2 == 0 else nc.scalar
                    q.dma_start_transpose(
                        buft[:, :, a * 128 : (a + 1) * 128], xg[:, :])

        # remaining slot fields: gate scale and OOB-masked output index
        gate_v = sinfoT[:, :, 2:3].rearrange("p a c -> p (a c)")
        occ_v = sinfoT[:, :, 3:4].rearrange("p a c -> p (a c)")
        oidx_f = rpool.tile([128, NS], FP, tag="oidxf")
        nc.vector.tensor_scalar(oidx_f, occ_v, -8192.0, 8192.0,
                                op0=OP.mult, op1=OP.add)
        nc.vector.tensor_add(oidx_f, oidx_f, tid_f)
        oidx = rpool.tile([128, NS], I32, tag="oidx")
        nc.vector.tensor_copy(oidx, oidx_f)

        # ----------------------- expert FFN -----------------------
        ht = mpool.tile([128, NF, CP], BF, tag="ht")        # hT[f%128, ft, slot]
        nc.vector.memset(ht[:, :, 1232:CP], 0.0)
        PRE = 6
        with tc.tile_pool(name="w1p", bufs=3) as w1p, tc.tile_pool(
            name="psh", bufs=2, space="PSUM"
        ) as psh:
            # mini-pass: slot-chunk 0 for the first PRE f-tiles starts as soon
            # as the first 4 slot tiles are gathered (fills the gather gap)
            for ff in range(PRE):
                w1s = w1p.tile([128, ND, 128], BF, tag="w1")
                nc.sync.dma_start(w1s, w1r[ff : ff + 1])
                ph = psh.tile([128, 512], FP, tag="h0", name="h0")
                for dt in range(ND):
                    nc.tensor.matmul(
                        ph,
                        lhsT=w1s[:, dt : dt + 1, :],
                        rhs=buft[:, dt : dt + 1, 0:512],
                        start=(dt == 0),
                        stop=(dt == ND - 1),
                    )
                nc.scalar.activation(
                    ht[:, ff : ff + 1, 0:512], ph, AF.Gelu_apprx_tanh,
                    bias=b1t_s[:, ff : ff + 1], scale=1.0)
            for ff in range(NF):
                chs = list(enumerate(CCH))[1:] if ff < PRE else list(enumerate(CCH))
                w1s = w1p.tile([128, ND, 128], BF, tag="w1")
                nc.sync.dma_start(w1s, w1r[ff : ff + 1])
                phs = {i: psh.tile([128, cw], FP, tag=f"h{i}", name=f"h{i}")
                       for i, (c0, cw) in chs}
                for dt in range(ND):
                    for i, (c0, cw) in chs:
                        nc.tensor.matmul(
                            phs[i],
                            lhsT=w1s[:, dt : dt + 1, :],
                            rhs=buft[:, dt : dt + 1, c0 : c0 + cw],
                            start=(dt == 0),
                            stop=(dt == ND - 1),
                        )
                for i, (c0, cw) in chs:
                    nc.scalar.activation(
                        ht[:, ff : ff + 1, c0 : c0 + cw],
                        phs[i],
                        AF.Gelu_apprx_tanh,
                        bias=b1t_s[:, ff : ff + 1],
                        scale=1.0,
                    )

        # matmul2 in y[slot, d] orientation: h tiles stationary, w2 moving
        with tc.tile_pool(name="psy", bufs=3, space="PSUM") as psy, tc.tile_pool(
            name="ys", bufs=2
        ) as ysp:
            for a in range(NS):
                yst = ysp.tile([128, D], FP, tag="ys")
                pys = [psy.tile([128, dw], FP, tag=f"y{i}", name=f"y{i}")
                       for i, (d0, dw) in enumerate(DCH)]
                for ft in range(NF):
                    for i, (d0, dw) in enumerate(DCH):
                        nc.tensor.matmul(
                            pys[i],
                            lhsT=ht[:, ft : ft + 1, a * 128 : (a + 1) * 128],
                            rhs=w2_s[:, ft : ft + 1, d0 : d0 + dw],
                            start=(ft == 0),
                            stop=False,
                        )
                for i, (d0, dw) in enumerate(DCH):
                    nc.tensor.matmul(
                        pys[i],
                        lhsT=onesb_s,
                        rhs=b2r_s[:, d0 : d0 + dw],
                        start=False,
                        stop=True,
                    )
                    nc.scalar.activation(
                        yst[:, d0 : d0 + dw], pys[i], AF.Identity,
                        bias=0.0, scale=gate_v[:, a : a + 1])
                nc.gpsimd.indirect_dma_start(
                    out=outd[:, :],
                    out_offset=IndirectOffsetOnAxis(ap=oidx[:, a : a + 1], axis=0),
                    in_=yst,
                    in_offset=None,
                    bounds_check=T - 1,
                    oob_is_err=False,
                )
    nc.compile()
    return nc


_NC = None


def _consts(eid):
    k = np.arange(128)
    tri = (k[:, None] <= k[None, :]).astype(np.float32)
    k64 = np.arange(64)
    tx64 = (k64[:, None] < k64[None, :]).astype(np.float32)
    ident = np.eye(128, dtype=np.float32)
    onesc = np.ones((128, 1), np.float32)
    onesb = np.ones((1, 128), ml_dtypes.bfloat16)
    eidr = np.full((128, 1), float(eid), np.float32)
    iota = np.broadcast_to(
        np.arange(1, CAP + 1, dtype=np.float32)[None, :], (128, CAP))
    iota = np.ascontiguousarray(iota)
    # (hi, lo, gate=0, one) per assignment chunk; tid = (c%32)*128 + p
    datc = np.zeros((128, KT, 4), np.float32)
    p = k[:, None]
    c = np.arange(KT)[None, :]
    tid = (c % NT) * 128 + p
    datc[:, :, 0] = tid // 64
    datc[:, :, 1] = tid % 64
    datc[:, :, 3] = 1.0
    datc = datc.reshape(128, KT * 4).astype(ml_dtypes.bfloat16)
    return tri, tx64, ident, onesc, onesb, eidr, iota, datc


def kernel(x, Wg, bg, W1, b1, W2, b2, _trace=False):
    global _NC
    if _NC is None:
        _NC = build_module()
    nc = _NC
    xt = np.ascontiguousarray(np.asarray(x, np.float32).reshape(T, D))
    xTm = np.ascontiguousarray(xt.T)
    xhm = xt.astype(ml_dtypes.bfloat16)
    Wg = np.asarray(Wg, np.float32)
    bgcm = np.ascontiguousarray(np.asarray(bg, np.float32)[:, None])
    in_maps = []
    for cc in range(E):
        tri, tx64, ident, onesc, onesb, eidr, iota, datc = _consts(cc)
        b1tc = np.ascontiguousarray(np.asarray(b1[cc], np.float32).reshape(NF, 128).T)
        b2rc = np.asarray(b2[cc], np.float32)[None, :].astype(ml_dtypes.bfloat16)
        in_maps.append({
            "xT": xTm, "xh": xhm, "wg": Wg, "bgc": bgcm,
            "w1": np.asarray(W1[cc], np.float32).astype(ml_dtypes.bfloat16),
            "b1t": b1tc,
            "w2": np.asarray(W2[cc], np.float32).astype(ml_dtypes.bfloat16),
            "b2r": np.ascontiguousarray(b2rc),
            "tri": tri, "tx64": tx64, "ident": ident, "onesc": onesc,
            "onesb": onesb, "eidr": eidr, "iota": iota, "datc": datc,
        })
    res = run_bass_kernel_spmd(nc, in_maps, core_ids=list(range(E)), trace=_trace)
    out = np.zeros((T, D), np.float32)
    for r in res.results:
        out += r["out"]
    kernel._last = res
    return out.reshape(4, 1024, D)


# revision 36
# speedup vs baseline: 1.1690x; 1.1690x over previous
"""MoE layer (top-2 of 8 experts, capacity 1229) on 8 Trainium2 NeuronCores.

Expert parallelism: core c owns expert c's FFN. The router (gate matmul +
top-2 + renormalized gates) is replicated on every core in fp32 (top-2
selection margins ~1.1e-5 require exact logits); it runs with Wg as the
PE-stationary operand and x^T streaming as the moving operand, producing
logits^T [8, T] which is PE-transposed back to token-major tiles.
Dispatch is fully on-chip: per-assignment slot positions via a
triangular-matmul cumsum, then a one-hot-matmul compaction builds the
per-slot (token, gate, occupancy) table — OH[p, s] = (St[p, c] == s+1)
from DVE is_equal in fp16 (St clamped to 2040 so all values stay
fp16-exact; positions > capacity match nothing, so overflow drops fall
out), then PE matmuls accumulate datc_c^T @ OH_c into a [4, slot] PSUM
table. Token ids ride as (hi, lo) base-64 digits so fp16 stays exact.
Tokens are gathered by row (indirect DMA, bf16) and transposed to
[d, slot] via the DMA XBAR transpose. The FFN runs bf16 matmuls; h stays
SBUF-resident so matmul2 accumulates over all 32 f-tiles at once, in
y[slot, d] orientation (h tiles stationary) so no output transpose is
needed; b2 is added via a K=1 ones-row matmul and the gate applied as a
per-partition ACT scale. Matmuls that reuse the previous stationary set
InstMatmult.ldweights=False to skip redundant PE weight loads. Each core
scatters its gate-scaled rows to its partial output; the host sums the 8
partials.
"""
import contextlib
import sys

sys.path.insert(0, "/opt/trn_rl_repo")

import ml_dtypes
import numpy as np

import concourse.bass as bass
import concourse.bacc as bacc
import concourse.mybir as mybir
from concourse.bass import IndirectOffsetOnAxis
from concourse.bass_utils import run_bass_kernel_spmd
from concourse.tile import TileContext

FP = mybir.dt.float32
BF = mybir.dt.bfloat16
F16 = mybir.dt.float16
I32 = mybir.dt.int32
U32 = mybir.dt.uint32
AF = mybir.ActivationFunctionType
OP = mybir.AluOpType

T, D, E, F = 4096, 1024, 8, 4096
CAP = 1229          # ceil(1.2 * T * 2 / 8)
CP = 1280           # padded slots (10 tiles)
NT = T // 128       # 32 token tiles
ND = D // 128       # 8 d tiles
NF = F // 32 // 4   # 32 f tiles
NS = CP // 128      # 10 slot tiles
KT = 64             # assignment chunks (2*T/128)
NCH = 8             # router token chunks of 512
OCH = [(0, 512), (512, 512), (1024, CAP - 1024)]   # one-hot windows (<=CAP)
CCH = [(0, 512), (512, 512), (1024, 256)]          # mm1 slot chunks (CP)
DCH = [(0, 512), (512, 512)]                       # mm2 d chunks


def _no_ldw(mm):
    mm.ins.ldweights = False
    return mm


def build_module():
    nc = bacc.Bacc(None, target_bir_lowering=False, debug=False)
    xT = nc.dram_tensor("xT", [D, T], FP, kind="ExternalInput")
    xh = nc.dram_tensor("xh", [T, D], BF, kind="ExternalInput")
    wg = nc.dram_tensor("wg", [D, E], FP, kind="ExternalInput")
    bgc = nc.dram_tensor("bgc", [E, 1], FP, kind="ExternalInput")
    w1 = nc.dram_tensor("w1", [D, F], BF, kind="ExternalInput")
    b1t = nc.dram_tensor("b1t", [128, NF], FP, kind="ExternalInput")
    w2 = nc.dram_tensor("w2", [F, D], BF, kind="ExternalInput")
    b2r = nc.dram_tensor("b2r", [1, D], BF, kind="ExternalInput")
    tri = nc.dram_tensor("tri", [128, 128], FP, kind="ExternalInput")
    tx64 = nc.dram_tensor("tx64", [64, 64], FP, kind="ExternalInput")
    ident = nc.dram_tensor("ident", [128, 128], FP, kind="ExternalInput")
    onesc = nc.dram_tensor("onesc", [128, 1], FP, kind="ExternalInput")
    onesb = nc.dram_tensor("onesb", [1, 128], BF, kind="ExternalInput")
    eidr = nc.dram_tensor("eidr", [128, 1], FP, kind="ExternalInput")
    iota = nc.dram_tensor("iota", [128, CAP], FP, kind="ExternalInput")
    datc = nc.dram_tensor("datc", [128, KT * 4], BF, kind="ExternalInput")
    outd = nc.dram_tensor("out", [T, D], FP, kind="ExternalOutput")

    xc_view = xT.rearrange("(dt p) t -> p dt t", p=128)
    w1r = w1.rearrange("(dt p) (ft j) -> ft p dt j", p=128, j=128)
    w2r = w2.rearrange("(ft p) d -> ft p d", p=128)
    wgr = wg.rearrange("(dt p) e -> p dt e", p=128)

    with TileContext(nc) as tc, contextlib.ExitStack() as ctx:
        cpool = ctx.enter_context(tc.tile_pool(name="consts", bufs=1))
        rpool = ctx.enter_context(tc.tile_pool(name="router", bufs=1))

        # sync queue: router weights then xT chunks (feeds the PE first)
        wg_s = cpool.tile([128, ND, E], FP, tag="wg")
        nc.sync.dma_start(wg_s, wgr[:, :, :])
        id_s = cpool.tile([128, 128], FP, tag="ident")
        nc.sync.dma_start(id_s, ident[:, :])
        # scalar queue: dispatch consts (needed ~60us in), then resident W2
        bgc_s = cpool.tile([E, 1], FP, tag="bgc")
        nc.scalar.dma_start(bgc_s, bgc[:, :])
        tri_s = cpool.tile([128, 128], FP, tag="tri")
        nc.scalar.dma_start(tri_s, tri[:, :])
        tx64_s = cpool.tile([64, 64], FP, tag="tx64")
        nc.scalar.dma_start(tx64_s, tx64[:, :])
        ones_s = cpool.tile([128, 1], FP, tag="ones")
        nc.scalar.dma_start(ones_s, onesc[:, :])
        onesb_s = cpool.tile([1, 128], BF, tag="onesb")
        nc.scalar.dma_start(onesb_s, onesb[:, :])
        eid_s = cpool.tile([128, 1], FP, tag="eidr")
        nc.scalar.dma_start(eid_s, eidr[:, :])
        iota_s = cpool.tile([128, CAP], FP, tag="iota")
        datc_s = cpool.tile([128, KT * 4], BF, tag="datc")
        b1t_s = cpool.tile([128, NF], FP, tag="b1t")
        b2r_s = cpool.tile([1, D], BF, tag="b2r")
        w2_s = cpool.tile([128, NF, D], BF, tag="w2")

        logits = rpool.tile([128, NT * E], FP, tag="logits")
        vals = rpool.tile([128, NT * E], FP, tag="vals")
        idxu = rpool.tile([128, NT * E], U32, tag="idxu")
        idxf = rpool.tile([128, NT * E], FP, tag="idxf")

        # ---- router: logits^T[e, t] = Wg^T @ x^T in fp32, then transpose ----
        with tc.tile_pool(name="xcp", bufs=3) as xcp, tc.tile_pool(
            name="ps8", bufs=2, space="PSUM"
        ) as ps8p, tc.tile_pool(name="ls8p", bufs=2) as ls8p, tc.tile_pool(
            name="pstr", bufs=2, space="PSUM"
        ) as pstr:
            for ch in range(NCH):
                xc = xcp.tile([128, ND, 512], FP, tag="xc")
                if ch == 0:
                    for k in range(4):
                        q = nc.sync if k % 2 == 0 else nc.scalar
                        q.dma_start(
                            xc[:, 2 * k : 2 * k + 2, :],
                            xc_view[:, 2 * k : 2 * k + 2, 0:512])
                else:
                    nc.sync.dma_start(
                        xc[:, 0:4, :], xc_view[:, 0:4, ch * 512 : (ch + 1) * 512])
                    nc.scalar.dma_start(
                        xc[:, 4:8, :], xc_view[:, 4:8, ch * 512 : (ch + 1) * 512])
                ps8 = ps8p.tile([E, 512], FP, tag="l8")
                for dt in range(ND):
                    nc.tensor.matmul(
                        ps8,
                        lhsT=wg_s[:, dt : dt + 1, :],
                        rhs=xc[:, dt : dt + 1, :],
                        start=(dt == 0),
                        stop=(dt == ND - 1),
                    )
                ls8 = ls8p.tile([E, 512], FP, tag="ls8")
                nc.scalar.activation(ls8, ps8, AF.Identity, bias=bgc_s, scale=1.0)
                for j in range(4):
                    tt = ch * 4 + j
                    pt8 = pstr.tile([128, E], FP, tag="t8")
                    nc.tensor.transpose(
                        pt8, ls8[:, j * 128 : (j + 1) * 128], id_s[0:E, 0:E])
                    lsl = logits[:, tt * E : (tt + 1) * E]
                    nc.vector.tensor_copy(lsl, pt8)
                    nc.vector.max(vals[:, tt * E : (tt + 1) * E], lsl)
                    nc.vector.max_index(idxu[:, tt * E : (tt + 1) * E],
                                        vals[:, tt * E : (tt + 1) * E], lsl)
        nc.vector.tensor_copy(idxf, idxu)

        # dispatch consts + resident FFN weights/biases: issued after the
        # router so the router's xT chunk streams get the queues first
        nc.scalar.dma_start(iota_s, iota[:, :])
        nc.scalar.dma_start(datc_s, datc[:, :])
        nc.scalar.dma_start(b1t_s, b1t[:, :])
        nc.scalar.dma_start(b2r_s, b2r[:, :])
        for ft in range(NF):
            nc.scalar.dma_start(w2_s[:, ft : ft + 1, :], w2r[ft : ft + 1])

        # gates: g2 = sigmoid(l2 - l1) = 0.5 + 0.5*tanh(0.5*(l2-l1)); g1 = 1-g2
        diff = rpool.tile([128, NT], FP, tag="diff")
        nc.vector.tensor_sub(diff, vals[:, 1::E], vals[:, 0::E])
        g2t = rpool.tile([128, NT], FP, tag="g2")
        nc.scalar.activation(g2t, diff, AF.Tanh, bias=0.0, scale=0.5)
        nc.vector.tensor_scalar(g2t, g2t, 0.5, 0.5, op0=OP.mult, op1=OP.add)
        g1t = rpool.tile([128, NT], FP, tag="g1")
        nc.vector.tensor_scalar(g1t, g2t, -1.0, 1.0, op0=OP.mult, op1=OP.add)

        # runtime gate columns into the (hi, lo, gate, one) chunk table
        dat = rpool.tile([128, KT * 4], BF, tag="dat")
        nc.vector.tensor_copy(dat, datc_s)
        nc.vector.tensor_copy(dat[:, 2 : NT * 4 : 4], g1t)
        nc.vector.tensor_copy(dat[:, NT * 4 + 2 :: 4], g2t)

        # ------------- dispatch: positions via cumsum matmuls -------------
        mpool = ctx.enter_context(tc.tile_pool(name="main", bufs=1))
        buft = mpool.tile([128, ND, CP], BF, tag="buft")
        sinfoT = rpool.tile([128, NS, 4], FP, tag="sinfoT")
        with tc.tile_pool(name="psc", bufs=1, space="PSUM") as psc, tc.tile_pool(
            name="ptd", bufs=1, space="PSUM"
        ) as ptd, tc.tile_pool(name="ohp", bufs=6) as ohp, tc.tile_pool(
            name="pst", bufs=2, space="PSUM"
        ) as pst:
            me = rpool.tile([128, 64], FP, tag="me")
            nc.vector.tensor_tensor(
                out=me[:, 0:NT], in0=idxf[:, 0::E],
                in1=eid_s.to_broadcast([128, NT]), op=OP.is_equal)
            nc.vector.tensor_tensor(
                out=me[:, NT:64], in0=idxf[:, 1::E],
                in1=eid_s.to_broadcast([128, NT]), op=OP.is_equal)
            ps_cs = psc.tile([128, 64], FP, tag="cs")
            nc.tensor.matmul(ps_cs, lhsT=tri_s, rhs=me, start=True, stop=False)
            ps_col = pst.tile([64, 1], FP, tag="col", bufs=1)
            nc.tensor.matmul(ps_col, lhsT=me, rhs=ones_s, start=True, stop=True)
            colv = rpool.tile([64, 1], FP, tag="colv")
            nc.vector.tensor_copy(colv, ps_col)
            colb = rpool.tile([64, 128], FP, tag="colb")
            nc.vector.tensor_copy(colb, colv.to_broadcast([64, 128]))
            # accumulate per-chunk base offsets onto the intra-chunk cumsum
            nc.tensor.matmul(ps_cs, lhsT=colb, rhs=tx64_s, start=False, stop=True)
            St = rpool.tile([128, 64], FP, tag="St")
            # mask: only matching assignments carry a slot position
            nc.vector.tensor_mul(St, ps_cs, me)

            # one-hot compaction: psum[4, slot] += datc_c^T @ (St_c == iota)
            ptds = [ptd.tile([4, 512], FP, tag=f"td{i}", name=f"td{i}")
                    for i in range(3)]
            for c in range(KT):
                oh = ohp.tile([128, CAP], BF, tag="oh")
                nc.vector.tensor_scalar(
                    oh, iota_s, St[:, c : c + 1], None, op0=OP.is_equal)
                for i, (c0, cw) in enumerate(OCH):
                    nc.tensor.matmul(
                        ptds[i][:, 0:cw],
                        lhsT=dat[:, 4 * c : 4 * c + 4],
                        rhs=oh[:, c0 : c0 + cw],
                        start=(c == 0),
                        stop=(c == KT - 1),
                    )
            sts = rpool.tile([4, CP], FP, tag="sts")
            nc.vector.memset(sts, 0.0)
            for i, (c0, cw) in enumerate(OCH):
                nc.vector.tensor_copy(sts[:, c0 : c0 + cw], ptds[i][:, 0:cw])
            # per slot tile: transpose [4, 128] -> [128, 4], extract the token
            # id, and kick off its gather + XBAR transpose immediately
            tid_f = rpool.tile([128, NS], FP, tag="tidf")
            tid_i = rpool.tile([128, NS], I32, tag="tidi")
            with tc.tile_pool(name="xg", bufs=3) as xgp:
                for a in range(NS):
                    pt = pst.tile([128, 4], FP, tag="t")
                    nc.tensor.transpose(
                        pt, sts[:, a * 128 : (a + 1) * 128], id_s[0:4, 0:4])
                    nc.vector.tensor_copy(sinfoT[:, a : a + 1, :], pt)
                    ta = tid_f[:, a : a + 1]
                    nc.vector.tensor_scalar(
                        ta, sinfoT[:, a : a + 1, 0:1].rearrange(
                            "p a c -> p (a c)"), 64.0, None, op0=OP.mult)
                    nc.vector.tensor_add(
                        ta, ta, sinfoT[:, a : a + 1, 1:2].rearrange(
                            "p a c -> p (a c)"))
                    nc.vector.tensor_copy(tid_i[:, a : a + 1], ta)
                    xg = xgp.tile([128, D], BF, tag="xg")
                    nc.gpsimd.indirect_dma_start(
                        out=xg,
                        out_offset=None,
                        in_=xh[:, :],
                        in_offset=IndirectOffsetOnAxis(
                            ap=tid_i[:, a : a + 1], axis=0),
                    )
                    q = nc.sync if a % 2 == 0 else nc.scalar
                    q.dma_start_transpose(
                        buft[:, :, a * 128 : (a + 1) * 128], xg[:, :])

        # remaining slot fields: gate scale and OOB-masked output index
        gate_v = sinfoT[:, :, 2:3].rearrange("p a c -> p (a c)")
        occ_v = sinfoT[:, :, 3:4].rearrange("p a c -> p (a c)")
        oidx_f = rpool.tile([128, NS], FP, tag="oidxf")
        nc.vector.tensor_scalar(oidx_f, occ_v, -8192.0, 8192.0,
                                op0=OP.mult, op1=OP.add)
        nc.vector.tensor_add(oidx_f, oidx_f, tid_f)
        oidx = rpool.tile([128, NS], I32, tag="oidx")
        nc.vector.tensor_copy(oidx, oidx_f)

        # ----------------------- expert FFN -----------------------
        ht = mpool.tile([128, NF, CP], BF, tag="ht")        # hT[f%128, ft, slot]
        PRE = 6
        with tc.tile_pool(name="w1p", bufs=3) as w1p, tc.tile_pool(
            name="psh", bufs=2, space="PSUM"
        ) as psh:
            # mini-pass: slot-chunk 0 for the first PRE f-tiles starts as soon
            # as the first 4 slot tiles are gathered (fills the gather gap)
            for ff in range(PRE):
                w1s = w1p.tile([128, ND, 128], BF, tag="w1")
                nc.sync.dma_start(w1s, w1r[ff : ff + 1])
                ph = psh.tile([128, 512], FP, tag="h0", name="h0")
                for dt in range(ND):
                    nc.tensor.matmul(
                        ph,
                        lhsT=w1s[:, dt : dt + 1, :],
                        rhs=buft[:, dt : dt + 1, 0:512],
                        start=(dt == 0),
                        stop=(dt == ND - 1),
                    )
                nc.scalar.activation(
                    ht[:, ff : ff + 1, 0:512], ph, AF.Gelu_apprx_tanh,
                    bias=b1t_s[:, ff : ff + 1], scale=1.0)
            for ff in range(NF):
                chs = list(enumerate(CCH))[1:] if ff < PRE else list(enumerate(CCH))
                w1s = w1p.tile([128, ND, 128], BF, tag="w1")
                nc.sync.dma_start(w1s, w1r[ff : ff + 1])
                phs = {i: psh.tile([128, cw], FP, tag=f"h{i}", name=f"h{i}")
                       for i, (c0, cw) in chs}
                for dt in range(ND):
                    for i, (c0, cw) in chs:
                        nc.tensor.matmul(
                            phs[i],
                            lhsT=w1s[:, dt : dt + 1, :],
                            rhs=buft[:, dt : dt + 1, c0 : c0 + cw],
                            start=(dt == 0),
                            stop=(dt == ND - 1),
                        )
                for i, (c0, cw) in chs:
                    nc.scalar.activation(
                        ht[:, ff : ff + 1, c0 : c0 + cw],
                        phs[i],
                        AF.Gelu_apprx_tanh,
                        bias=b1t_s[:, ff : ff + 1],
                        scale=1.0,
                    )

        # matmul2 in y[slot, d] orientation: h tiles stationary, w2 moving
        with tc.tile_pool(name="psy", bufs=3, space="PSUM") as psy, tc.tile_pool(
            name="ys", bufs=2
        ) as ysp:
            for a in range(NS):
                yst = ysp.tile([128, D], FP, tag="ys")
                pys = [psy.tile([128, dw], FP, tag=f"y{i}", name=f"y{i}")
                       for i, (d0, dw) in enumerate(DCH)]
                for ft in range(NF):
                    for i, (d0, dw) in enumerate(DCH):
                        nc.tensor.matmul(
                            pys[i],
                            lhsT=ht[:, ft : ft + 1, a * 128 : (a + 1) * 128],
                            rhs=w2_s[:, ft : ft + 1, d0 : d0 + dw],
                            start=(ft == 0),
                            stop=False,
                        )
                for i, (d0, dw) in enumerate(DCH):
                    nc.tensor.matmul(
                        pys[i],
                        lhsT=onesb_s,
                        rhs=b2r_s[:, d0 : d0 + dw],
                        start=False,
                        stop=True,
                    )
                    nc.scalar.activation(
                        yst[:, d0 : d0 + dw], pys[i], AF.Identity,
                        bias=0.0, scale=gate_v[:, a : a + 1])
                nc.gpsimd.indirect_dma_start(
                    out=outd[:, :],
                    out_offset=IndirectOffsetOnAxis(ap=oidx[:, a : a + 1], axis=0),
                    in_=yst,
                    in_offset=None,
                    bounds_check=T - 1,
                    oob_is_err=False,
                )
    nc.compile()
    return nc


_NC = None


def _consts(eid):
    k = np.arange(128)
    tri = (k[:, None] <= k[None, :]).astype(np.float32)
    k64 = np.arange(64)
    tx64 = (k64[:, None] < k64[None, :]).astype(np.float32)
    ident = np.eye(128, dtype=np.float32)
    onesc = np.ones((128, 1), np.float32)
    onesb = np.ones((1, 128), ml_dtypes.bfloat16)
    eidr = np.full((128, 1), float(eid), np.float32)
    iota = np.broadcast_to(
        np.arange(1, CAP + 1, dtype=np.float32)[None, :], (128, CAP))
    iota = np.ascontiguousarray(iota)
    # (hi, lo, gate=0, one) per assignment chunk; tid = (c%32)*128 + p
    datc = np.zeros((128, KT, 4), np.float32)
    p = k[:, None]
    c = np.arange(KT)[None, :]
    tid = (c % NT) * 128 + p
    datc[:, :, 0] = tid // 64
    datc[:, :, 1] = tid % 64
    datc[:, :, 3] = 1.0
    datc = datc.reshape(128, KT * 4).astype(ml_dtypes.bfloat16)
    return tri, tx64, ident, onesc, onesb, eidr, iota, datc


def kernel(x, Wg, bg, W1, b1, W2, b2, _trace=False):
    global _NC
    if _NC is None:
        _NC = build_module()
    nc = _NC
    xt = np.ascontiguousarray(np.asarray(x, np.float32).reshape(T, D))
    xTm = np.ascontiguousarray(xt.T)
    xhm = xt.astype(ml_dtypes.bfloat16)
    Wg = np.asarray(Wg, np.float32)
    bgcm = np.ascontiguousarray(np.asarray(bg, np.float32)[:, None])
    in_maps = []
    for cc in range(E):
        tri, tx64, ident, onesc, onesb, eidr, iota, datc = _consts(cc)
        b1tc = np.ascontiguousarray(np.asarray(b1[cc], np.float32).reshape(NF, 128).T)
        b2rc = np.asarray(b2[cc], np.float32)[None, :].astype(ml_dtypes.bfloat16)
        in_maps.append({
            "xT": xTm, "xh": xhm, "wg": Wg, "bgc": bgcm,
            "w1": np.asarray(W1[cc], np.float32).astype(ml_dtypes.bfloat16),
            "b1t": b1tc,
            "w2": np.asarray(W2[cc], np.float32).astype(ml_dtypes.bfloat16),
            "b2r": np.ascontiguousarray(b2rc),
            "tri": tri, "tx64": tx64, "ident": ident, "onesc": onesc,
            "onesb": onesb, "eidr": eidr, "iota": iota, "datc": datc,
        })
    res = run_bass_kernel_spmd(nc, in_maps, core_ids=list(range(E)), trace=_trace)
    out = np.zeros((T, D), np.float32)
    for r in res.results:
        out += r["out"]
    kernel._last = res
    return out.reshape(4, 1024, D)
